# revision 26
# baseline (speedup 1.0000x reference)
"""Trainium2 Bass kernel for CommutatorConv2d.

Math: with lambda_c=0, lambda_a=1 the reference is a conv2d with effective
kernel  w_eff[o,i,r,s] = krow[o,i,s] + kcol[o,i,r]  (krow = sum_r w, kcol =
sum_s w), plus bias.  That kernel lives in a 5-dim matrix subspace
(row-functions + col-functions share the constants), so the 9-tap conv
factors into FIVE contraction-128 matmuls per output tile:

  y[o,h,w] = W1[o,i] @ xbox[i,h,w]            (xbox = 3x3 box sum of x)
           + d0[o,i] @ xv[i,h,w-1] + d2[o,i] @ xv[i,h,w+1]
           + e0[o,i] @ xh[i,h-1,w] + e2[o,i] @ xh[i,h+1,w]  + bias[o]

  where xv/xh are vertical/horizontal 3-tap sums of zero-padded x,
  d0 = krow0-krow1, d2 = krow2-krow1, e0 = kcol0-kcol1, e2 = kcol2-kcol1,
  W1 = krow1+kcol1  (the center taps absorbed into the box-sum term).
  All spatial shifts are free access-pattern reads.  5 matmuls/tile
  instead of the 9 of direct conv or 6 of the two-1D-conv factorization.

Sharding: data-parallel over batch; 4 images per core on 8 cores.

Schedule notes (from neuron-profile traces):
- xbox ships precomputed from the host (+0.8MB/image DMA, the DMA stream
  has slack) so the DVE only builds xv and xh: 4 adds per image, well
  under the PE's per-image budget.  Computing xbox on-device (6 adds)
  makes the DVE the critical resource through the first two images; the
  GpSimd engine is no help (its first tensor op measured 10us).
- All DMAs issue from the sync queue in priority order (head chunks,
  weights, bias, remaining chunks): the DMA engines drain one queue's
  descriptors in order, so the transfers that gate the first matmul
  complete first.
- Dummy matmuls bridge the tensor engine from the framework preamble to
  the first real tile with no idle gap; any gap decays the HAM p-state /
  utilization limit and costs a re-ramp over real work.
- Box-sum adds are emitted per DMA chunk so tiles unlock as data
  arrives; matmul order per tile follows readiness (xv, xbox, xh).
  Each image's half-1 tail tiles are DEFERRED into the next image's PE
  stream as guaranteed-ready filler that absorbs any box-sum transient.
- Output is stored as bf16 (host upcasts): halves store traffic, and the
  final half-image ends in two 4-row tiles whose stores issue on two
  different queues so the kernel tail only waits on one 224-column DMA.
"""

import os
import numpy as np
import ml_dtypes

import concourse.bass as bass
import concourse.bacc as bacc
import concourse.mybir as mybir
import concourse.tile as tile
from concourse.bass_utils import run_bass_kernel_spmd

B, CI, CO, H, W = 32, 128, 256, 56, 56
NCORES = 8
BPC = B // NCORES          # images per core
HP, WP = H + 2, W + 2      # padded spatial dims
NPIX = H * W               # 3136
ROWT = 8                   # output rows per matmul tile
NT = H // ROWT             # 7 pixel tiles per image
NTILE = ROWT * W           # 448 columns per matmul
NTAP = 5                   # matmuls per tile

ROW_CHUNKS0 = [10, 26, 42, HP]  # image-0 row chunks; chunk to row r unlocks tiles t with 8t+10 <= r
ROW_CHUNKS = [26, HP]       # later images: 2 chunks so tiles t0-t2 unlock early
N_WARM = 28                 # PE warmup matmuls (bridge idle->real work, keeps HAM limit up)
WARMC = 448                 # dummy-matmul tile width

F32 = mybir.dt.float32
BF16 = mybir.dt.bfloat16


def build_nc():
    nc = bacc.Bacc(None, enable_partition_id=False)
    xin = nc.declare_dram_parameter("xp", [BPC, CI, HP, WP], BF16, isOutput=False)
    xbin = nc.declare_dram_parameter("xbx", [BPC, CI, H, WP], BF16, isOutput=False)
    wk = nc.declare_dram_parameter("klhs", [CI, NTAP, 2, 128], BF16, isOutput=False)
    bb = nc.declare_dram_parameter("bias2", [CI, 2], F32, isOutput=False)
    y = nc.declare_dram_parameter("y", [BPC, CO, H, W], BF16, isOutput=True)

    xflat = xin.rearrange("b c h w -> b c (h w)")
    xbflat = xbin.rearrange("b c h w -> b c (h w)")
    yflat = y.rearrange("b o h w -> b o (h w)")
    wkflat = wk.rearrange("i t h o -> i (t h o)")
    NPAD = HP * WP           # 3364
    NV = H * WP              # 3248 (rows 0..55 of padded, all 58 cols)
    NW = NTAP * 128          # weight columns per half

    with tile.TileContext(nc) as tc:
        with (
            tc.tile_pool(name="const", bufs=1) as cpool,
            tc.tile_pool(name="xp", bufs=2) as xpool,
            tc.tile_pool(name="xv", bufs=2) as vpool,
            tc.tile_pool(name="xh", bufs=2) as hpool,
            tc.tile_pool(name="xb", bufs=2) as bpool,
            tc.tile_pool(name="yo", bufs=4) as ypool,
            tc.tile_pool(name="ps", bufs=7, space="PSUM") as pspool,
        ):
            klhs_sb = cpool.tile([CI, 2 * NW], BF16)
            bias_sb = cpool.tile([CI, 2], F32)
            kl4 = klhs_sb.rearrange("i (t h o) -> i t h o", h=2, o=128)

            # PE warmup: dummy matmuls issued while the first input DMAs are
            # in flight keep the tensor engine active so the HAM utilization
            # limit ramp overlaps the DMA wait instead of the real matmuls.
            # pad_mm emits more of them INSIDE the early real tile stream:
            # any sub-us PE gap there breaks the 3.4us continuous-busy window
            # the HAM clock-gate needs, deferring full clock by several us.
            warm = cpool.tile([128, WARMC], BF16)
            nc.gpsimd.memset(warm[:], 0.0)
            warm_ps = pspool.tile([128, WARMC], F32, bufs=1, tag="warm")

            def pad_mm(n, cols=WARMC):
                for _ in range(n):
                    nc.tensor.matmul(
                        warm_ps[:, 0:cols], warm[:, 0:128], warm[:, 0:cols],
                        start=True, stop=True,
                    )

            pad_mm(N_WARM, cols=128)

            deferred = []  # emit-closures for the previous image's tail tiles

            for b in range(BPC):
                row_chunks = ROW_CHUNKS0 if b == 0 else ROW_CHUNKS

                xp_sb = xpool.tile([CI, NPAD], BF16)
                xb = bpool.tile([CI, NV], BF16)
                xp3d = xflat[b].rearrange("i (h w) -> i h w", w=WP)
                xps3 = xp_sb.rearrange("i (h w) -> i h w", w=WP)
                xb3d = xbflat[b].rearrange("i (h w) -> i h w", w=WP)
                xbs3 = xb.rearrange("i (h w) -> i h w", w=WP)
                # two hardware-DGE queues: DMA_DIRECT2D issue costs ~0.6us of
                # queue time apiece, so a single queue serializes the head
                # loads past the point the PE needs them.  The DMA engines
                # round-robin across the queues' descriptors, so the head
                # transfers are interleaved by DEADLINE, alternating queues:
                # urgent small loads must not share the engines with bulk
                # ones.  The scalar queue is idle until the first activation.
                if b == 0:
                    # the head loads go on ONE queue in strict deadline
                    # order: the DMA engines' per-stream completion tails
                    # blow up 2-3us whenever transfers interleave, so the
                    # critical sequence must have the engines exclusively.
                    # (A second queue only parallelizes the ~0.6us per-DMA
                    # issue cost - not worth the tail latency on the head.)
                    r0 = 0
                    for ci, r1 in enumerate(row_chunks):
                        nc.sync.dma_start(
                            out=xps3[:, r0:r1, :], in_=xp3d[:, r0:r1, :]
                        )
                        if ci == 0:
                            nc.sync.dma_start(out=klhs_sb[:], in_=wkflat[:])
                            nc.sync.dma_start(
                                out=xbs3[:, 0 : r1 - 2, :], in_=xb3d[:, 0 : r1 - 2, :]
                            )
                            nc.sync.dma_start(out=bias_sb[:], in_=bb[:])
                        else:
                            nc.sync.dma_start(
                                out=xbs3[:, r0 - 2 : r1 - 2, :],
                                in_=xb3d[:, r0 - 2 : r1 - 2, :],
                            )
                        r0 = r1
                else:
                    r0 = 0
                    for r1 in row_chunks:
                        nc.sync.dma_start(
                            out=xps3[:, r0:r1, :], in_=xp3d[:, r0:r1, :]
                        )
                        v0b, v1b = (0 if r0 == 0 else r0 - 2), (H if r1 == HP else r1 - 2)
                        nc.scalar.dma_start(
                            out=xbs3[:, v0b:v1b, :], in_=xb3d[:, v0b:v1b, :]
                        )
                        r0 = r1

                # box-sums, emitted per DMA chunk so they overlap the loads:
                # xv[j] = xp[j] + xp[j+58] + xp[j+116]   (rows 0..55)
                # xh[j] = xp[j] + xp[j+1] + xp[j+2]      (rows 0..57, garbage
                #                                         at cols 56/57 unused)
                xvt = vpool.tile([CI, NV], BF16)
                xv = vpool.tile([CI, NV], BF16)
                xht = hpool.tile([CI, NPAD], BF16)
                xh = hpool.tile([CI, NPAD], BF16)
                v0 = h0r = 0
                for s1 in row_chunks:
                    v1 = H if s1 == HP else s1 - 2    # xv rows ready
                    h1 = s1                           # xh rows ready
                    a, z = v0 * WP, v1 * WP
                    nc.vector.tensor_add(
                        xvt[:, a:z], xp_sb[:, a:z], xp_sb[:, a + WP : z + WP]
                    )
                    nc.vector.tensor_add(
                        xv[:, a:z], xvt[:, a:z], xp_sb[:, a + 2 * WP : z + 2 * WP]
                    )
                    a, z = h0r * WP, h1 * WP - 2
                    nc.vector.tensor_add(
                        xht[:, a:z], xp_sb[:, a:z], xp_sb[:, a + 1 : z + 1]
                    )
                    nc.vector.tensor_add(
                        xh[:, a:z], xht[:, a:z], xp_sb[:, a + 2 : z + 2]
                    )
                    v0, h0r = v1, h1

                xv3 = xv.rearrange("i (h w) -> i h w", w=WP)   # [128, 56, 58]
                xh3 = xh.rearrange("i (h w) -> i h w", w=WP)   # [128, 58, 58]

                youts = {}

                def mm5(ps, half, h0, nr, pads=(0, 0),
                        kl4=kl4, xv3=xv3, xh3=xh3, xbs3=xbs3):
                    # tap order follows data readiness: xbox (pure DMA, no
                    # vector dep), xv (DVE ops 1-2), xh (DVE ops 3-4).  pads
                    # keep the PE busy across the first tile's sem waits.
                    nc.tensor.matmul(
                        ps[:], kl4[:, 4, half, :],
                        xbs3[:, h0 : h0 + nr, 0:W], start=True, stop=False,
                    )
                    pad_mm(pads[0], cols=128)
                    nc.tensor.matmul(
                        ps[:], kl4[:, 0, half, :],
                        xv3[:, h0 : h0 + nr, 0:W], start=False, stop=False,
                    )
                    nc.tensor.matmul(
                        ps[:], kl4[:, 1, half, :],
                        xv3[:, h0 : h0 + nr, 2 : 2 + W], start=False, stop=False,
                    )
                    pad_mm(pads[1], cols=128)
                    nc.tensor.matmul(
                        ps[:], kl4[:, 2, half, :],
                        xh3[:, h0 : h0 + nr, 0:W], start=False, stop=False,
                    )
                    nc.tensor.matmul(
                        ps[:], kl4[:, 3, half, :],
                        xh3[:, h0 + 2 : h0 + 2 + nr, 0:W], start=False, stop=True,
                    )

                def emit(half, t, pads=(0, 0), b=b, youts=youts, mm5=mm5):
                    if half not in youts:
                        youts[half] = ypool.tile(
                            [128, NPIX], BF16, name=f"yout_{b}_{half}", tag="yout"
                        )
                    yout = youts[half]
                    h0 = t * ROWT
                    ps = pspool.tile([128, NTILE], F32, name=f"ps_{b}_{half}_{t}", tag="ps")
                    mm5(ps, half, h0, ROWT, pads=pads)
                    nc.scalar.activation(
                        yout[:, t * NTILE : (t + 1) * NTILE],
                        ps[:],
                        mybir.ActivationFunctionType.Identity,
                        bias=bias_sb[:, half : half + 1],
                    )
                    last_block = b == BPC - 1 and half == 1
                    if t == 3:
                        nc.sync.dma_start(
                            out=yflat[b, half * 128 : half * 128 + 128, 0 : 4 * NTILE],
                            in_=yout[:, 0 : 4 * NTILE],
                        )
                    elif t >= 4 and last_block:
                        # final block: per-tile stores so the kernel tail
                        # only waits on small DMAs
                        nc.sync.dma_start(
                            out=yflat[
                                b,
                                half * 128 : half * 128 + 128,
                                t * NTILE : (t + 1) * NTILE,
                            ],
                            in_=yout[:, t * NTILE : (t + 1) * NTILE],
                        )
                    if t == NT - 1 and not last_block:
                        nc.sync.dma_start(
                            out=yflat[b, half * 128 : half * 128 + 128, 4 * NTILE : NPIX],
                            in_=yout[:, 4 * NTILE : NPIX],
                        )

                # the previous image's deferred tiles are guaranteed-ready PE
                # filler: they run while this image's box-sum chain finishes
                for fn in deferred:
                    fn()
                deferred = []

                if b == 0:
                    # image 0: interleave halves so each arriving row chunk
                    # immediately unlocks two tiles of PE work.  Dummy-matmul
                    # padding between the first groups bridges the box-sum
                    # chain's latency with PE-busy time instead of gaps.
                    emit(0, 0, pads=(2, 2))
                    emit(1, 0)
                    pad_mm(2)
                    for t in range(1, 4):
                        emit(0, t)
                        emit(1, t)
                    for t in range(4, NT):
                        emit(0, t)
                    early = []
                    defer = [(1, t) for t in range(4, NT)]
                elif b < BPC - 1:
                    early = [(0, t) for t in range(NT)] + [(1, t) for t in range(4)]
                    defer = [(1, t) for t in range(4, NT)]
                else:
                    # last image: stop half 1 before its final 8-row tile;
                    # rows 48-55 are emitted below as two 4-row tiles so the
                    # kernel tail waits on a 224-column activation + store
                    early = [(0, t) for t in range(NT)] + [(1, t) for t in range(NT - 1)]
                    defer = []
                for half, t in early:
                    emit(half, t)
                deferred = [
                    (lambda half=half, t=t, emit=emit: emit(half, t))
                    for half, t in defer
                ]

                if b == BPC - 1:
                    yout1 = youts[1]
                    for k in range(2):
                        h0 = (NT - 1) * ROWT + 4 * k
                        c0 = h0 * W
                        ps = pspool.tile([128, 4 * W], F32, name=f"ps_tail_{k}", tag="ps")
                        mm5(ps, 1, h0, 4)
                        nc.scalar.activation(
                            yout1[:, c0 : c0 + 4 * W], ps[:],
                            mybir.ActivationFunctionType.Identity,
                            bias=bias_sb[:, 1:2],
                        )
                        # two queues: the second store's issue overlaps the
                        # first's so the tail waits on one 224-col transfer
                        eng = nc.gpsimd if k == 0 else nc.sync
                        eng.dma_start(
                            out=yflat[b, 128:256, c0 : c0 + 4 * W],
                            in_=yout1[:, c0 : c0 + 4 * W],
                        )

            # read the warm PSUM bank at the very end so the warmup matmuls
            # are never dead-code-eliminated but gate nothing
            warm_out = cpool.tile([128, 32], F32)
            nc.scalar.activation(
                warm_out[:], warm_ps[:, 0:32], mybir.ActivationFunctionType.Copy
            )
    nc.finalize()
    return nc


_NC_CACHE = {}


def _get_nc():
    if "nc" not in _NC_CACHE:
        _NC_CACHE["nc"] = build_nc()
    return _NC_CACHE["nc"]


def make_in_maps(x, weight, bias):
    x = np.asarray(x, dtype=np.float32)
    weight = np.asarray(weight, dtype=np.float32)
    bias = np.asarray(bias, dtype=np.float32)

    krow = weight.sum(axis=3)  # [O, I, 3]
    kcol = weight.sum(axis=2)  # [O, I, 3]
    taps = [
        krow[:, :, 0] - krow[:, :, 1],   # d0 @ xv(w-1)
        krow[:, :, 2] - krow[:, :, 1],   # d2 @ xv(w+1)
        kcol[:, :, 0] - kcol[:, :, 1],   # e0 @ xh(h-1)
        kcol[:, :, 2] - kcol[:, :, 1],   # e2 @ xh(h+1)
        krow[:, :, 1] + kcol[:, :, 1],   # W1 @ xbox
    ]
    klhs = np.empty((CI, NTAP, 2, 128), np.float32)
    for half in range(2):
        o0 = half * 128
        for t, tap in enumerate(taps):
            klhs[:, t, half, :] = tap[o0 : o0 + 128, :].T
    klhs = klhs.astype(ml_dtypes.bfloat16)

    xp = np.zeros((B, CI, HP, WP), np.float32)
    xp[:, :, 1 : H + 1, 1 : W + 1] = x

    # 3x3 box sum, laid out [B, CI, 56 rows, 58 cols] with cols 56/57 unused
    xv = xp[:, :, 0:H, :] + xp[:, :, 1 : H + 1, :] + xp[:, :, 2 : H + 2, :]
    xbx = np.zeros((B, CI, H, WP), np.float32)
    xbx[:, :, :, 0:W] = xv[:, :, :, 0:W] + xv[:, :, :, 1 : W + 1] + xv[:, :, :, 2 : W + 2]

    xp = xp.astype(ml_dtypes.bfloat16)
    xbx = xbx.astype(ml_dtypes.bfloat16)

    bias2 = np.ascontiguousarray(bias.reshape(2, 128).T)  # [128, 2] f32

    return [
        {
            "xp": xp[c * BPC : (c + 1) * BPC],
            "xbx": xbx[c * BPC : (c + 1) * BPC],
            "klhs": klhs,
            "bias2": bias2,
        }
        for c in range(NCORES)
    ]


def run(in_maps, **kwargs):
    nc = _get_nc()
    return run_bass_kernel_spmd(nc, in_maps, list(range(NCORES)), **kwargs)


def kernel(x, weight, bias):
    res = run(make_in_maps(x, weight, bias))
    return np.concatenate(
        [res.results[c]["y"].astype(np.float32) for c in range(NCORES)], axis=0
    )


# revision 27
# speedup vs baseline: 1.0717x; 1.0717x over previous
"""Trainium2 Bass kernel for CommutatorConv2d.

Math: with lambda_c=0, lambda_a=1 the reference is a conv2d with effective
kernel  w_eff[o,i,r,s] = krow[o,i,s] + kcol[o,i,r]  (krow = sum_r w, kcol =
sum_s w), plus bias.  That kernel lives in a 5-dim matrix subspace
(row-functions + col-functions share the constants), so the 9-tap conv
factors into FIVE contraction-128 matmuls per output tile:

  y[o,h,w] = W1[o,i] @ xbox[i,h,w]            (xbox = 3x3 box sum of x)
           + d0[o,i] @ xv[i,h,w-1] + d2[o,i] @ xv[i,h,w+1]
           + e0[o,i] @ xh[i,h-1,w] + e2[o,i] @ xh[i,h+1,w]  + bias[o]

  where xv/xh are vertical/horizontal 3-tap sums of zero-padded x,
  d0 = krow0-krow1, d2 = krow2-krow1, e0 = kcol0-kcol1, e2 = kcol2-kcol1,
  W1 = krow1+kcol1  (the center taps absorbed into the box-sum term).
  All spatial shifts are free access-pattern reads.  5 matmuls/tile
  instead of the 9 of direct conv or 6 of the two-1D-conv factorization.

Sharding: data-parallel over batch; 4 images per core on 8 cores.

Schedule notes (from neuron-profile traces):
- xbox ships precomputed from the host (+0.8MB/image DMA, the DMA stream
  has slack) so the DVE only builds xv and xh: 4 adds per image, well
  under the PE's per-image budget.  Computing xbox on-device (6 adds)
  makes the DVE the critical resource through the first two images; the
  GpSimd engine is no help (its first tensor op measured 10us).
- All DMAs issue from the sync queue in priority order (head chunks,
  weights, bias, remaining chunks): the DMA engines drain one queue's
  descriptors in order, so the transfers that gate the first matmul
  complete first.
- Dummy matmuls bridge the tensor engine from the framework preamble to
  the first real tile with no idle gap; any gap decays the HAM p-state /
  utilization limit and costs a re-ramp over real work.
- Box-sum adds are emitted per DMA chunk so tiles unlock as data
  arrives; matmul order per tile follows readiness (xv, xbox, xh).
  Each image's half-1 tail tiles are DEFERRED into the next image's PE
  stream as guaranteed-ready filler that absorbs any box-sum transient.
- Output is stored as bf16 (host upcasts): halves store traffic, and the
  final half-image ends in two 4-row tiles whose stores issue on two
  different queues so the kernel tail only waits on one 224-column DMA.
"""

import os
import numpy as np
import ml_dtypes

import concourse.bass as bass
import concourse.bacc as bacc
import concourse.mybir as mybir
import concourse.tile as tile
from concourse.bass_utils import run_bass_kernel_spmd

B, CI, CO, H, W = 32, 128, 256, 56, 56
NCORES = 8
BPC = B // NCORES          # images per core
HP, WP = H + 2, W + 2      # padded spatial dims
NPIX = H * W               # 3136
ROWT = 8                   # output rows per matmul tile
NT = H // ROWT             # 7 pixel tiles per image
NTILE = ROWT * W           # 448 columns per matmul
NTAP = 5                   # matmuls per tile

ROW_CHUNKS0 = [10, 26, 42, HP]  # image-0 row chunks; chunk to row r unlocks tiles t with 8t+10 <= r
ROW_CHUNKS = [26, HP]       # later images: 2 chunks so tiles t0-t2 unlock early
N_WARM = 28                 # PE warmup matmuls (bridge idle->real work, keeps HAM limit up)
WARMC = 448                 # dummy-matmul tile width

F32 = mybir.dt.float32
BF16 = mybir.dt.bfloat16


def build_nc():
    nc = bacc.Bacc(None, enable_partition_id=False)
    xin = nc.declare_dram_parameter("xp", [BPC, CI, HP, WP], BF16, isOutput=False)
    xbin = nc.declare_dram_parameter("xbx", [BPC, CI, H, WP], BF16, isOutput=False)
    wk = nc.declare_dram_parameter("klhs", [CI, NTAP, 2, 128], BF16, isOutput=False)
    bb = nc.declare_dram_parameter("bias2", [CI, 2], F32, isOutput=False)
    y = nc.declare_dram_parameter("y", [BPC, CO, H, W], BF16, isOutput=True)

    xflat = xin.rearrange("b c h w -> b c (h w)")
    xbflat = xbin.rearrange("b c h w -> b c (h w)")
    yflat = y.rearrange("b o h w -> b o (h w)")
    wkflat = wk.rearrange("i t h o -> i (t h o)")
    NPAD = HP * WP           # 3364
    NV = H * WP              # 3248 (rows 0..55 of padded, all 58 cols)
    NW = NTAP * 128          # weight columns per half

    with tile.TileContext(nc) as tc:
        with (
            tc.tile_pool(name="const", bufs=1) as cpool,
            tc.tile_pool(name="xp", bufs=2) as xpool,
            tc.tile_pool(name="xv", bufs=2) as vpool,
            tc.tile_pool(name="xh", bufs=2) as hpool,
            tc.tile_pool(name="xb", bufs=2) as bpool,
            tc.tile_pool(name="yo", bufs=4) as ypool,
            tc.tile_pool(name="ps", bufs=7, space="PSUM") as pspool,
        ):
            klhs_sb = cpool.tile([CI, 2 * NW], BF16)
            bias_sb = cpool.tile([CI, 2], F32)
            kl4 = klhs_sb.rearrange("i (t h o) -> i t h o", h=2, o=128)

            # PE warmup: dummy matmuls issued while the first input DMAs are
            # in flight keep the tensor engine active so the HAM utilization
            # limit ramp overlaps the DMA wait instead of the real matmuls.
            # pad_mm emits more of them INSIDE the early real tile stream:
            # any sub-us PE gap there breaks the 3.4us continuous-busy window
            # the HAM clock-gate needs, deferring full clock by several us.
            warm = cpool.tile([128, WARMC], BF16)
            nc.gpsimd.memset(warm[:], 0.0)
            warm_ps = pspool.tile([128, WARMC], F32, bufs=1, tag="warm")

            def pad_mm(n, cols=WARMC):
                for _ in range(n):
                    nc.tensor.matmul(
                        warm_ps[:, 0:cols], warm[:, 0:128], warm[:, 0:cols],
                        start=True, stop=True,
                    )

            pad_mm(N_WARM, cols=128)

            deferred = []  # emit-closures for the previous image's tail tiles

            for b in range(BPC):
                row_chunks = ROW_CHUNKS0 if b == 0 else ROW_CHUNKS

                xp_sb = xpool.tile([CI, NPAD], BF16)
                xb = bpool.tile([CI, NV], BF16)
                xp3d = xflat[b].rearrange("i (h w) -> i h w", w=WP)
                xps3 = xp_sb.rearrange("i (h w) -> i h w", w=WP)
                xb3d = xbflat[b].rearrange("i (h w) -> i h w", w=WP)
                xbs3 = xb.rearrange("i (h w) -> i h w", w=WP)
                # two hardware-DGE queues: DMA_DIRECT2D issue costs ~0.6us of
                # queue time apiece, so a single queue serializes the head
                # loads past the point the PE needs them.  The DMA engines
                # round-robin across the queues' descriptors, so the head
                # transfers are interleaved by DEADLINE, alternating queues:
                # urgent small loads must not share the engines with bulk
                # ones.  The scalar queue is idle until the first activation.
                if b == 0:
                    # the head loads go on ONE queue in strict deadline
                    # order: the DMA engines' per-stream completion tails
                    # blow up 2-3us whenever transfers interleave, so the
                    # critical sequence must have the engines exclusively.
                    # (A second queue only parallelizes the ~0.6us per-DMA
                    # issue cost - not worth the tail latency on the head.)
                    r0 = 0
                    for ci, r1 in enumerate(row_chunks):
                        nc.sync.dma_start(
                            out=xps3[:, r0:r1, :], in_=xp3d[:, r0:r1, :]
                        )
                        if ci == 0:
                            nc.sync.dma_start(out=klhs_sb[:], in_=wkflat[:])
                            nc.sync.dma_start(
                                out=xbs3[:, 0 : r1 - 2, :], in_=xb3d[:, 0 : r1 - 2, :]
                            )
                            nc.sync.dma_start(out=bias_sb[:], in_=bb[:])
                        else:
                            nc.sync.dma_start(
                                out=xbs3[:, r0 - 2 : r1 - 2, :],
                                in_=xb3d[:, r0 - 2 : r1 - 2, :],
                            )
                        r0 = r1
                else:
                    # later images stay on the same queue: a second queue
                    # issues its (dependency-free) DMAs immediately at kernel
                    # start and the transfers trample the critical head loads
                    r0 = 0
                    for r1 in row_chunks:
                        nc.sync.dma_start(
                            out=xps3[:, r0:r1, :], in_=xp3d[:, r0:r1, :]
                        )
                        v0b, v1b = (0 if r0 == 0 else r0 - 2), (H if r1 == HP else r1 - 2)
                        nc.sync.dma_start(
                            out=xbs3[:, v0b:v1b, :], in_=xb3d[:, v0b:v1b, :]
                        )
                        r0 = r1

                # box-sums, emitted per DMA chunk so they overlap the loads:
                # xv[j] = xp[j] + xp[j+58] + xp[j+116]   (rows 0..55)
                # xh[j] = xp[j] + xp[j+1] + xp[j+2]      (rows 0..57, garbage
                #                                         at cols 56/57 unused)
                xvt = vpool.tile([CI, NV], BF16)
                xv = vpool.tile([CI, NV], BF16)
                xht = hpool.tile([CI, NPAD], BF16)
                xh = hpool.tile([CI, NPAD], BF16)
                v0 = h0r = 0
                for s1 in row_chunks:
                    v1 = H if s1 == HP else s1 - 2    # xv rows ready
                    h1 = s1                           # xh rows ready
                    a, z = v0 * WP, v1 * WP
                    nc.vector.tensor_add(
                        xvt[:, a:z], xp_sb[:, a:z], xp_sb[:, a + WP : z + WP]
                    )
                    nc.vector.tensor_add(
                        xv[:, a:z], xvt[:, a:z], xp_sb[:, a + 2 * WP : z + 2 * WP]
                    )
                    a, z = h0r * WP, h1 * WP - 2
                    nc.vector.tensor_add(
                        xht[:, a:z], xp_sb[:, a:z], xp_sb[:, a + 1 : z + 1]
                    )
                    nc.vector.tensor_add(
                        xh[:, a:z], xht[:, a:z], xp_sb[:, a + 2 : z + 2]
                    )
                    v0, h0r = v1, h1

                xv3 = xv.rearrange("i (h w) -> i h w", w=WP)   # [128, 56, 58]
                xh3 = xh.rearrange("i (h w) -> i h w", w=WP)   # [128, 58, 58]

                youts = {}

                def mm5(ps, half, h0, nr, pads=(0, 0),
                        kl4=kl4, xv3=xv3, xh3=xh3, xbs3=xbs3):
                    # tap order follows data readiness: xbox (pure DMA, no
                    # vector dep), xv (DVE ops 1-2), xh (DVE ops 3-4).  pads
                    # keep the PE busy across the first tile's sem waits.
                    nc.tensor.matmul(
                        ps[:], kl4[:, 4, half, :],
                        xbs3[:, h0 : h0 + nr, 0:W], start=True, stop=False,
                    )
                    pad_mm(pads[0], cols=128)
                    nc.tensor.matmul(
                        ps[:], kl4[:, 0, half, :],
                        xv3[:, h0 : h0 + nr, 0:W], start=False, stop=False,
                    )
                    nc.tensor.matmul(
                        ps[:], kl4[:, 1, half, :],
                        xv3[:, h0 : h0 + nr, 2 : 2 + W], start=False, stop=False,
                    )
                    pad_mm(pads[1], cols=128)
                    nc.tensor.matmul(
                        ps[:], kl4[:, 2, half, :],
                        xh3[:, h0 : h0 + nr, 0:W], start=False, stop=False,
                    )
                    nc.tensor.matmul(
                        ps[:], kl4[:, 3, half, :],
                        xh3[:, h0 + 2 : h0 + 2 + nr, 0:W], start=False, stop=True,
                    )

                def emit(half, t, pads=(0, 0), b=b, youts=youts, mm5=mm5):
                    if half not in youts:
                        youts[half] = ypool.tile(
                            [128, NPIX], BF16, name=f"yout_{b}_{half}", tag="yout"
                        )
                    yout = youts[half]
                    h0 = t * ROWT
                    ps = pspool.tile([128, NTILE], F32, name=f"ps_{b}_{half}_{t}", tag="ps")
                    mm5(ps, half, h0, ROWT, pads=pads)
                    nc.scalar.activation(
                        yout[:, t * NTILE : (t + 1) * NTILE],
                        ps[:],
                        mybir.ActivationFunctionType.Identity,
                        bias=bias_sb[:, half : half + 1],
                    )
                    last_block = b == BPC - 1 and half == 1
                    if t == 3:
                        nc.sync.dma_start(
                            out=yflat[b, half * 128 : half * 128 + 128, 0 : 4 * NTILE],
                            in_=yout[:, 0 : 4 * NTILE],
                        )
                    elif t >= 4 and last_block:
                        # final block: per-tile stores so the kernel tail
                        # only waits on small DMAs
                        nc.sync.dma_start(
                            out=yflat[
                                b,
                                half * 128 : half * 128 + 128,
                                t * NTILE : (t + 1) * NTILE,
                            ],
                            in_=yout[:, t * NTILE : (t + 1) * NTILE],
                        )
                    if t == NT - 1 and not last_block:
                        nc.sync.dma_start(
                            out=yflat[b, half * 128 : half * 128 + 128, 4 * NTILE : NPIX],
                            in_=yout[:, 4 * NTILE : NPIX],
                        )

                # the previous image's deferred tiles are guaranteed-ready PE
                # filler: they run while this image's box-sum chain finishes
                for fn in deferred:
                    fn()
                deferred = []

                if b == 0:
                    # image 0: interleave halves so each arriving row chunk
                    # immediately unlocks two tiles of PE work.  Dummy-matmul
                    # padding between the first groups bridges the box-sum
                    # chain's latency with PE-busy time instead of gaps.
                    emit(0, 0, pads=(2, 2))
                    emit(1, 0)
                    pad_mm(2)
                    for t in range(1, 4):
                        emit(0, t)
                        emit(1, t)
                    for t in range(4, NT):
                        emit(0, t)
                    early = []
                    defer = [(1, t) for t in range(4, NT)]
                elif b < BPC - 1:
                    early = [(0, t) for t in range(NT)] + [(1, t) for t in range(4)]
                    defer = [(1, t) for t in range(4, NT)]
                else:
                    # last image: stop half 1 before its final 8-row tile;
                    # rows 48-55 are emitted below as two 4-row tiles so the
                    # kernel tail waits on a 224-column activation + store
                    early = [(0, t) for t in range(NT)] + [(1, t) for t in range(NT - 1)]
                    defer = []
                for half, t in early:
                    emit(half, t)
                deferred = [
                    (lambda half=half, t=t, emit=emit: emit(half, t))
                    for half, t in defer
                ]

                if b == BPC - 1:
                    yout1 = youts[1]
                    for k in range(2):
                        h0 = (NT - 1) * ROWT + 4 * k
                        c0 = h0 * W
                        ps = pspool.tile([128, 4 * W], F32, name=f"ps_tail_{k}", tag="ps")
                        mm5(ps, 1, h0, 4)
                        nc.scalar.activation(
                            yout1[:, c0 : c0 + 4 * W], ps[:],
                            mybir.ActivationFunctionType.Identity,
                            bias=bias_sb[:, 1:2],
                        )
                        # two queues: the second store's issue overlaps the
                        # first's so the tail waits on one 224-col transfer
                        eng = nc.gpsimd if k == 0 else nc.sync
                        eng.dma_start(
                            out=yflat[b, 128:256, c0 : c0 + 4 * W],
                            in_=yout1[:, c0 : c0 + 4 * W],
                        )

            # read the warm PSUM bank at the very end so the warmup matmuls
            # are never dead-code-eliminated but gate nothing
            warm_out = cpool.tile([128, 32], F32)
            nc.scalar.activation(
                warm_out[:], warm_ps[:, 0:32], mybir.ActivationFunctionType.Copy
            )
    nc.finalize()
    return nc


_NC_CACHE = {}


def _get_nc():
    if "nc" not in _NC_CACHE:
        _NC_CACHE["nc"] = build_nc()
    return _NC_CACHE["nc"]


def make_in_maps(x, weight, bias):
    x = np.asarray(x, dtype=np.float32)
    weight = np.asarray(weight, dtype=np.float32)
    bias = np.asarray(bias, dtype=np.float32)

    krow = weight.sum(axis=3)  # [O, I, 3]
    kcol = weight.sum(axis=2)  # [O, I, 3]
    taps = [
        krow[:, :, 0] - krow[:, :, 1],   # d0 @ xv(w-1)
        krow[:, :, 2] - krow[:, :, 1],   # d2 @ xv(w+1)
        kcol[:, :, 0] - kcol[:, :, 1],   # e0 @ xh(h-1)
        kcol[:, :, 2] - kcol[:, :, 1],   # e2 @ xh(h+1)
        krow[:, :, 1] + kcol[:, :, 1],   # W1 @ xbox
    ]
    klhs = np.empty((CI, NTAP, 2, 128), np.float32)
    for half in range(2):
        o0 = half * 128
        for t, tap in enumerate(taps):
            klhs[:, t, half, :] = tap[o0 : o0 + 128, :].T
    klhs = klhs.astype(ml_dtypes.bfloat16)

    xp = np.zeros((B, CI, HP, WP), np.float32)
    xp[:, :, 1 : H + 1, 1 : W + 1] = x

    # 3x3 box sum, laid out [B, CI, 56 rows, 58 cols] with cols 56/57 unused
    xv = xp[:, :, 0:H, :] + xp[:, :, 1 : H + 1, :] + xp[:, :, 2 : H + 2, :]
    xbx = np.zeros((B, CI, H, WP), np.float32)
    xbx[:, :, :, 0:W] = xv[:, :, :, 0:W] + xv[:, :, :, 1 : W + 1] + xv[:, :, :, 2 : W + 2]

    xp = xp.astype(ml_dtypes.bfloat16)
    xbx = xbx.astype(ml_dtypes.bfloat16)

    bias2 = np.ascontiguousarray(bias.reshape(2, 128).T)  # [128, 2] f32

    return [
        {
            "xp": xp[c * BPC : (c + 1) * BPC],
            "xbx": xbx[c * BPC : (c + 1) * BPC],
            "klhs": klhs,
            "bias2": bias2,
        }
        for c in range(NCORES)
    ]


def run(in_maps, **kwargs):
    nc = _get_nc()
    return run_bass_kernel_spmd(nc, in_maps, list(range(NCORES)), **kwargs)


def kernel(x, weight, bias):
    res = run(make_in_maps(x, weight, bias))
    return np.concatenate(
        [res.results[c]["y"].astype(np.float32) for c in range(NCORES)], axis=0
    )
